# revision 17
# baseline (speedup 1.0000x reference)
"""Bass/Tile TRN2 kernel for a 10-layer tanh-RNN discriminator.

Reference computation:
  x: (T=512, B=128, H=100) f32
  10 stacked RNN layers: h_t = tanh(W_ih @ x_t + b_ih + b_hh + W_hh @ h_{t-1})
  final: sigmoid(W_lin @ h + b_lin) -> (T*B,)

Strategy: data-parallel over batch across 8 cores (16 samples/core), RNN
weights replicated.  Per core, all 10 layers run as a layer-staggered
wavefront (stagger = 16 links = 2 blocks of 8 timesteps), so at every
link all active layers share the same block-step/parity and one fused
tanh ACT per PSUM-bank-aligned layer chunk (0-3 / 4-7 / 8-9) covers its
layers.  Matmul operands are fp16 (PE runs fp32 as two half-speed
passes; fp16 is one) padded to K=M=128 to trigger Fast Weight Load;
PSUM accumulation stays fp32 and only the layer-0 input GEMM (on raw x)
is fp32, so end-to-end error stays ~1e-4.  Each layer's input GEMM
(W_ih, batched over an 8-step block) runs off the critical chain into a
DVE-zeroed PSUM region with start=False accumulation throughout --
start=True clears has_written BANK-wide and would clobber other layers'
in-flight accumulations.  Biases ride in weight row 100 against a
maintained h row 100 == 1.0 (weight[100,100] == 20.0, tanh(20) == 1.0
regenerates it), which also feeds the final linear's b_lin.  The output
linear + sigmoid runs per finished layer-9 block via
sigmoid(z) = 0.5*(1+tanh(z/2)) to stay on one ACT table set.
"""

from contextlib import ExitStack

import numpy as np

import concourse.bass as bass
import concourse.mybir as mybir
import concourse.tile as tile
from concourse.bass_utils import run_bass_kernel_spmd

# ---------------------------------------------------------------------------
# Workaround: this walrus build rejects sem waits on Drain
# (setupSyncWait<NEURON_ISA_TPB_CTRL_NO_STRUCT>: "Too many sync wait
# commands").  TileContext's exit attaches end-of-kernel DMA-queue waits to
# the SP Drain; move them onto NOP carriers (one wait each) before draining.


def _patched_drain_and_barrier(self, tick_clock, wait_clock):
    nc = self.nc
    carrier = nc.sync.nop(nofuse=True, hint="drain_wait_carrier")
    wait_clock.add_sem_waits(
        carrier.ins, tile.ScopedClock({None: tick_clock.global_clock})
    )
    si = carrier.ins.sync_info
    waits = list(si.on_wait) if si is not None else []
    if len(waits) > 1:
        carrier.ins.sync_info = mybir.SyncInfo(on_wait=[waits[0]], on_update=[])
        for w in waits[1:]:
            extra = nc.sync.nop(nofuse=True, hint="drain_wait_carrier")
            extra.ins.sync_info = mybir.SyncInfo(on_wait=[w], on_update=[])

    nc.sync.drain()
    nc.all_engine_barrier()
    assert self.sems is not None
    popped = nc._tile_sem_poison_stack.pop()
    assert popped is self._sem_poison
    nc.clear_and_free_semaphores(list(self.sems.allocated().values()))
    nc.all_engine_barrier()


tile.TileContext._drain_and_barrier = _patched_drain_and_barrier

# This walrus build also caps sync waits per compute instruction below what
# Tile emits (observed: Activation/Matmult with 3 waits rejected).  Cap every
# instruction at MAXW waits; hoist the excess onto same-engine NOP carriers
# inserted immediately before, preserving per-engine program order.
_MAXW = 1
_waitnop_counter = [0]


def _split_excess_waits(nc):
    for fn in nc.m.functions:
        for bb in fn.blocks:
            insts = list(bb.instructions)
            out = []
            changed = False
            for inst in insts:
                si = inst.sync_info
                waits = list(si.on_wait) if si is not None else []
                if len(waits) > _MAXW:
                    changed = True
                    extra, keep = waits[:-_MAXW], waits[-_MAXW:]
                    for i in range(0, len(extra), _MAXW):
                        _waitnop_counter[0] += 1
                        out.append(
                            mybir.InstNoOp(
                                name=f"waitnop_{_waitnop_counter[0]}",
                                engine=inst.engine,
                                sync_info=mybir.SyncInfo(
                                    on_wait=extra[i:i + _MAXW], on_update=[]
                                ),
                                bass_nofuse=True,
                            )
                        )
                    inst.sync_info = mybir.SyncInfo(
                        on_wait=keep, on_update=list(si.on_update)
                    )
                out.append(inst)
            if changed:
                bb.instructions = out

# ---------------------------------------------------------------------------

F32 = mybir.dt.float32
F16 = mybir.dt.float16
TANH = mybir.ActivationFunctionType.Tanh

H = 100          # hidden size
K = 101          # contraction with ones-row (bias folding)
L = 10           # layers
B = 128          # global batch
NCORES = 8
BC = B // NCORES  # 16 samples per core
TB = 8           # timesteps per block
STAG = 2 * TB    # layer stagger in links (2 blocks)
NXCH = 8         # x is loaded in 8 chunk DMAs

_BUILD_CACHE = {}


def _build(T, split_waits=True):
    assert T % (NXCH * TB) == 0
    n_links = T + STAG * (L - 1)
    cols = T * BC           # per-core activation columns (t*BC + b)
    xch = cols // NXCH      # columns per x chunk
    KP = 128                # padded contraction dim (enables FWL for fp16)
    NB = TB * BC            # columns per block (128)

    nc = bass.Bass("TRN2", target_bir_lowering=False, debug=False)
    x_d = nc.dram_tensor("x", [KP, cols], F32, kind="ExternalInput").ap()
    wih0_d = nc.dram_tensor("wih0", [KP, KP], F32, kind="ExternalInput").ap()
    wihr_d = nc.dram_tensor("wihr", [L - 1, KP, KP], F16, kind="ExternalInput").ap()
    whh_d = nc.dram_tensor("whh", [L, KP, KP], F16, kind="ExternalInput").ap()
    wlin_d = nc.dram_tensor("wlin", [KP, 1], F16, kind="ExternalInput").ap()
    out_d = nc.dram_tensor("out", [1, cols], F32, kind="ExternalOutput").ap()

    # ACT/PSUM chunks: layer groups aligned to PSUM banks (4 layers = 1 bank)
    CHUNKS = [(0, 4), (4, 8), (8, 10)]

    def chunk_of(l):
        for ci, (a, b) in enumerate(CHUNKS):
            if a <= l < b:
                return ci, l - a
        raise AssertionError

    with ExitStack() as ctx:
        tc = ctx.enter_context(tile.TileContext(nc))
        sing = ctx.enter_context(tc.tile_pool(name="sing", bufs=1))
        psum = ctx.enter_context(tc.tile_pool(name="psum", bufs=1, space="PSUM"))

        # persistent SBUF
        xt = [sing.tile([KP, xch], F32, name=f"xt{k}", tag=f"x{k}")
              for k in range(NXCH)]
        wih0 = sing.tile([KP, KP], F32)
        wihr = sing.tile([KP, (L - 1) * KP], F16)
        whh = sing.tile([KP, L * KP], F16)
        wlin = sing.tile([KP, 1], F16)
        # h block buffers, one per chunk: rows 0-99 h, row 100 == 1.0
        # (tanh(20), feeds every bias row incl. the final linear's b_lin),
        # rows 101-127 == 0
        hb = [sing.tile([KP, (b - a) * 2 * NB], F16, name=f"hb{ci}")
              for ci, (a, b) in enumerate(CHUNKS)]
        outs = sing.tile([1, 4 * NB], F32)

        # PSUM pre-activation accumulators: per chunk, per block parity --
        # chunk c's TANH reads never share a PSUM tile/bank with another
        # chunk's matmul writes (no false serialization)
        pre = [[psum.tile([KP, (b - a) * NB], F32, name=f"pre{ci}_{q}")
                for q in range(2)]
               for ci, (a, b) in enumerate(CHUNKS)]
        lg = psum.tile([1, 2 * NB], F32)

        pre_v = [[p.rearrange("p (l s x) -> p l s x", l=b - a, s=TB)
                  for p in pre[ci]]
                 for ci, (a, b) in enumerate(CHUNKS)]
        hb_v = [t.rearrange("p (l q s x) -> p l q s x", l=b - a, q=2, s=TB)
                for t, (a, b) in zip(hb, CHUNKS)]

        # ---- prologue: loads ----
        for k in range(NXCH):
            nc.gpsimd.dma_start(out=xt[k][:], in_=x_d[:, k * xch:(k + 1) * xch])
        nc.gpsimd.dma_start(out=wih0[:], in_=wih0_d[:])
        nc.gpsimd.dma_start(
            out=wihr.rearrange("p (l j) -> p l j", l=L - 1),
            in_=wihr_d.rearrange("l p j -> p l j"),
        )
        nc.gpsimd.dma_start(
            out=whh.rearrange("p (l j) -> p l j", l=L),
            in_=whh_d.rearrange("l p j -> p l j"),
        )
        nc.gpsimd.dma_start(out=wlin[:], in_=wlin_d[:])
        # rows 96-99 are overwritten by the first ACT writes; 100-127 stay 0
        # until the first ACT sets row 100 = tanh(20) = 1.0
        for t in hb:
            nc.vector.memset(t[96:KP, :], 0.0)
        # pre-zero PSUM once so first accumulates never meet NaN garbage
        for ci in range(len(CHUNKS)):
            for q in range(2):
                nc.vector.memset(pre[ci][q][:], 0.0)

        def gemm(l, t0):
            # input GEMM for layer l's block starting at t0.  Biases ride in
            # weight row 100 against h row 100 == 1.0; weight element
            # [100,100] == 20.0 regenerates the row (tanh(20) == 1.0).
            # start=False everywhere: start=True would clear has_written
            # BANK-wide, clobbering other layers in-flight; after the DVE
            # zero-fill, plain-write-or-add are both correct.
            q = (t0 // TB) % 2
            ci, li = chunk_of(l)
            outp = pre_v[ci][q][0:KP, li, :, :]
            nc.vector.memset(outp, 0.0)
            if l == 0:
                c0 = t0 * BC
                ch, off = c0 // xch, c0 % xch
                rhs = xt[ch][0:KP, off:off + NB]
                lhsT = wih0[:]
            else:
                pci, pli = chunk_of(l - 1)
                rhs = hb_v[pci][0:KP, pli, q, :, :]
                lhsT = wihr[:, (l - 1) * KP:l * KP]
            nc.tensor.matmul(outp, lhsT, rhs, start=False, stop=False,
                             skip_group_check=True)

        def mm_step(l, g):
            t = g - STAG * l
            if t == 0:
                return
            step, parity = g % TB, (g // TB) % 2
            pt = t - 1
            ci, li = chunk_of(l)
            h_src = hb_v[ci][0:KP, li, (pt // TB) % 2, pt % TB, :]
            outp = pre_v[ci][parity][0:KP, li, step, :]
            nc.tensor.matmul(outp, whh[:, l * KP:(l + 1) * KP], h_src,
                             start=False, stop=(step == TB - 1 or t == T - 1),
                             skip_group_check=True)

        def act_chunk(ci, ls, g):
            step, parity = g % TB, (g // TB) % 2
            a0 = CHUNKS[ci][0]
            a, b = ls[0] - a0, ls[-1] - a0
            src = pre_v[ci][parity][0:KP, a:b + 1, step, :]
            dst = hb_v[ci][0:KP, a:b + 1, parity, step, :]
            nc.scalar.activation(dst, src, TANH)

        def final_block(g):
            t9 = g - STAG * (L - 1)
            t0 = t9 - (TB - 1)
            q = (g // TB) % 2
            ci, li = chunk_of(L - 1)
            nc.tensor.matmul(lg[0:1, q * NB:(q + 1) * NB], wlin[:, 0:1],
                             hb_v[ci][0:KP, li, q, :, :], start=True, stop=True,
                             skip_group_check=True)
            # sigmoid(z) = 0.5*(1 + tanh(z/2)) -- stays on the tanh table set
            nc.scalar.activation(outs[0:1, q * NB:(q + 1) * NB],
                                 lg[0:1, q * NB:(q + 1) * NB], TANH, scale=0.5)
            nc.vector.tensor_scalar(outs[0:1, (2 + q) * NB:(3 + q) * NB],
                                    outs[0:1, q * NB:(q + 1) * NB],
                                    0.5, 0.5,
                                    mybir.AluOpType.mult, mybir.AluOpType.add)
            nc.gpsimd.dma_start(out=out_d[0:1, t0 * BC:t0 * BC + NB],
                                in_=outs[0:1, (2 + q) * NB:(3 + q) * NB])

        gemm(0, 0)

        for g in range(n_links):
            lmax = min(L - 1, g // STAG)
            lmin = max(0, -(-(g - (T - 1)) // STAG))
            active = list(range(lmin, lmax + 1))

            for ci in reversed(range(len(CHUNKS))):
                a, b = CHUNKS[ci]
                ls = [l for l in active if a <= l < b]
                if not ls:
                    continue
                for l in ls:
                    mm_step(l, g)
                act_chunk(ci, ls, g)
                if ci == len(CHUNKS) - 1 and L - 1 in ls and g % TB == TB - 1:
                    final_block(g)

            # off-chain input GEMMs for upcoming blocks, spread across links
            for l in range(L):
                s_l = 2 + (l % 6)
                if g % TB == s_l % TB:
                    t0 = g + TB - s_l - STAG * l
                    if 0 <= t0 <= T - TB and not (l == 0 and t0 == 0):
                        gemm(l, t0)

    nc._dbg = {"hb": hb, "pre": pre, "whh": whh, "xt": xt,
               "outs": outs, "lg": lg}
    if split_waits:
        _split_excess_waits(nc)
    return nc


def _get(T):
    if T not in _BUILD_CACHE:
        _BUILD_CACHE[T] = _build(T)
    return _BUILD_CACHE[T]


def _prep(x, W_ih, W_hh, b_ih, b_hh, W_lin, b_lin):
    T = x.shape[0]
    KP = 128
    bsum = (b_ih + b_hh).astype(np.float32)      # (L, H)
    wih0 = np.zeros((KP, KP), np.float32)
    wih0[0:H, 0:H] = W_ih[0].T
    wih0[H, 0:H] = bsum[0]
    wih0[H, H] = 20.0        # tanh(20) == 1.0 -> regenerates h row 100
    wihr = np.zeros((L - 1, KP, KP), np.float16)
    wihr[:, 0:H, 0:H] = W_ih[1:].transpose(0, 2, 1)
    wihr[:, H, 0:H] = bsum[1:]
    wihr[:, H, H] = 20.0
    whh = np.zeros((L, KP, KP), np.float16)
    whh[:, 0:H, 0:H] = W_hh.transpose(0, 2, 1)
    wlin = np.zeros((KP, 1), np.float16)
    wlin[0:H, 0] = W_lin[0]
    wlin[H, 0] = b_lin[0]
    in_maps = []
    for c in range(NCORES):
        xc = x[:, c * BC:(c + 1) * BC, :]          # (T, 16, 100)
        xct = xc.transpose(2, 0, 1).reshape(H, T * BC)
        xa = np.zeros((KP, T * BC), dtype=np.float32)
        xa[0:H] = xct
        xa[H] = 1.0
        in_maps.append({"x": xa, "wih0": wih0, "wihr": wihr,
                        "whh": whh, "wlin": wlin})
    return in_maps


def _run(inputs, trace=False, **kw):
    x = np.asarray(inputs["x"], dtype=np.float32)
    T = x.shape[0]
    nc = _get(T)
    in_maps = _prep(
        x,
        np.asarray(inputs["W_ih"], np.float32),
        np.asarray(inputs["W_hh"], np.float32),
        np.asarray(inputs["b_ih"], np.float32),
        np.asarray(inputs["b_hh"], np.float32),
        np.asarray(inputs["W_lin"], np.float32),
        np.asarray(inputs["b_lin"], np.float32),
    )
    res = run_bass_kernel_spmd(nc, in_maps, core_ids=list(range(NCORES)),
                               trace=trace, **kw)
    out = np.empty((T, B), dtype=np.float32)
    for c in range(NCORES):
        out[:, c * BC:(c + 1) * BC] = res.results[c]["out"].reshape(T, BC)
    return out.reshape(-1), res


def kernel(**inputs):
    out, _ = _run(inputs, trace=False)
    return out


# revision 18
# speedup vs baseline: 1.0382x; 1.0382x over previous
"""Bass/Tile TRN2 kernel for a 10-layer tanh-RNN discriminator.

Reference computation:
  x: (T=512, B=128, H=100) f32
  10 stacked RNN layers: h_t = tanh(W_ih @ x_t + b_ih + b_hh + W_hh @ h_{t-1})
  final: sigmoid(W_lin @ h + b_lin) -> (T*B,)

Strategy: data-parallel over batch across 8 cores (16 samples/core), RNN
weights replicated.  Per core, all 10 layers run as a layer-staggered
wavefront (stagger = 16 links = 2 blocks of 8 timesteps), so at every
link all active layers share the same block-step/parity and one fused
tanh ACT per PSUM-bank-aligned layer chunk (0-3 / 4-7 / 8-9) covers its
layers.  Matmul operands are fp16 (PE runs fp32 as two half-speed
passes; fp16 is one) padded to K=M=128 to trigger Fast Weight Load;
PSUM accumulation stays fp32 and only the layer-0 input GEMM (on raw x)
is fp32, so end-to-end error stays ~1e-4.  Each layer's input GEMM
(W_ih, batched over an 8-step block) runs off the critical chain into a
DVE-zeroed PSUM region with start=False accumulation throughout --
start=True clears has_written BANK-wide and would clobber other layers'
in-flight accumulations.  Biases ride in weight row 100 against a
maintained h row 100 == 1.0 (weight[100,100] == 20.0, tanh(20) == 1.0
regenerates it), which also feeds the final linear's b_lin.  The output
linear + sigmoid runs per finished layer-9 block via
sigmoid(z) = 0.5*(1+tanh(z/2)) to stay on one ACT table set.
"""

from contextlib import ExitStack

import numpy as np

import concourse.bass as bass
import concourse.mybir as mybir
import concourse.tile as tile
from concourse.bass_utils import run_bass_kernel_spmd

# ---------------------------------------------------------------------------
# Workaround: this walrus build rejects sem waits on Drain
# (setupSyncWait<NEURON_ISA_TPB_CTRL_NO_STRUCT>: "Too many sync wait
# commands").  TileContext's exit attaches end-of-kernel DMA-queue waits to
# the SP Drain; move them onto NOP carriers (one wait each) before draining.


def _patched_drain_and_barrier(self, tick_clock, wait_clock):
    nc = self.nc
    carrier = nc.sync.nop(nofuse=True, hint="drain_wait_carrier")
    wait_clock.add_sem_waits(
        carrier.ins, tile.ScopedClock({None: tick_clock.global_clock})
    )
    si = carrier.ins.sync_info
    waits = list(si.on_wait) if si is not None else []
    if len(waits) > 1:
        carrier.ins.sync_info = mybir.SyncInfo(on_wait=[waits[0]], on_update=[])
        for w in waits[1:]:
            extra = nc.sync.nop(nofuse=True, hint="drain_wait_carrier")
            extra.ins.sync_info = mybir.SyncInfo(on_wait=[w], on_update=[])

    nc.sync.drain()
    nc.all_engine_barrier()
    assert self.sems is not None
    popped = nc._tile_sem_poison_stack.pop()
    assert popped is self._sem_poison
    nc.clear_and_free_semaphores(list(self.sems.allocated().values()))
    nc.all_engine_barrier()


tile.TileContext._drain_and_barrier = _patched_drain_and_barrier

# This walrus build also caps sync waits per compute instruction below what
# Tile emits (observed: Activation/Matmult with 3 waits rejected).  Cap every
# instruction at MAXW waits; hoist the excess onto same-engine NOP carriers
# inserted immediately before, preserving per-engine program order.
_MAXW = 1
_waitnop_counter = [0]


def _split_excess_waits(nc):
    for fn in nc.m.functions:
        for bb in fn.blocks:
            insts = list(bb.instructions)
            out = []
            changed = False
            for inst in insts:
                si = inst.sync_info
                waits = list(si.on_wait) if si is not None else []
                if len(waits) > _MAXW:
                    changed = True
                    extra, keep = waits[:-_MAXW], waits[-_MAXW:]
                    for i in range(0, len(extra), _MAXW):
                        _waitnop_counter[0] += 1
                        out.append(
                            mybir.InstNoOp(
                                name=f"waitnop_{_waitnop_counter[0]}",
                                engine=inst.engine,
                                sync_info=mybir.SyncInfo(
                                    on_wait=extra[i:i + _MAXW], on_update=[]
                                ),
                                bass_nofuse=True,
                            )
                        )
                    inst.sync_info = mybir.SyncInfo(
                        on_wait=keep, on_update=list(si.on_update)
                    )
                out.append(inst)
            if changed:
                bb.instructions = out

# ---------------------------------------------------------------------------

F32 = mybir.dt.float32
F16 = mybir.dt.float16
TANH = mybir.ActivationFunctionType.Tanh

H = 100          # hidden size
K = 101          # contraction with ones-row (bias folding)
L = 10           # layers
B = 128          # global batch
NCORES = 8
BC = B // NCORES  # 16 samples per core
TB = 8           # timesteps per block
STAG = 2 * TB    # layer stagger in links (2 blocks)
NXCH = 8         # x is loaded in 8 chunk DMAs

_BUILD_CACHE = {}


def _build(T, split_waits=True):
    assert T % (NXCH * TB) == 0
    n_links = T + STAG * (L - 1)
    cols = T * BC           # per-core activation columns (t*BC + b)
    xch = cols // NXCH      # columns per x chunk
    KP = 128                # padded contraction dim (enables FWL for fp16)
    NB = TB * BC            # columns per block (128)

    nc = bass.Bass("TRN2", target_bir_lowering=False, debug=False)
    x_d = nc.dram_tensor("x", [KP, cols], F32, kind="ExternalInput").ap()
    wih0_d = nc.dram_tensor("wih0", [KP, KP], F32, kind="ExternalInput").ap()
    wihr_d = nc.dram_tensor("wihr", [L - 1, KP, KP], F16, kind="ExternalInput").ap()
    whh_d = nc.dram_tensor("whh", [L, KP, KP], F16, kind="ExternalInput").ap()
    wlin_d = nc.dram_tensor("wlin", [KP, 1], F16, kind="ExternalInput").ap()
    out_d = nc.dram_tensor("out", [1, cols], F32, kind="ExternalOutput").ap()

    # ACT/PSUM chunks: layer groups aligned to PSUM banks (4 layers = 1 bank)
    CHUNKS = [(0, 4), (4, 8), (8, 10)]

    def chunk_of(l):
        for ci, (a, b) in enumerate(CHUNKS):
            if a <= l < b:
                return ci, l - a
        raise AssertionError

    with ExitStack() as ctx:
        tc = ctx.enter_context(tile.TileContext(nc))
        sing = ctx.enter_context(tc.tile_pool(name="sing", bufs=1))
        psum = ctx.enter_context(tc.tile_pool(name="psum", bufs=1, space="PSUM"))

        # persistent SBUF
        xt = [sing.tile([KP, xch], F32, name=f"xt{k}", tag=f"x{k}")
              for k in range(NXCH)]
        wih0 = sing.tile([KP, KP], F32)
        wihr = sing.tile([KP, (L - 1) * KP], F16)
        whh = sing.tile([KP, L * KP], F16)
        wlin = sing.tile([KP, 1], F16)
        # h block buffers, one per chunk: rows 0-99 h, row 100 == 1.0
        # (tanh(20), feeds every bias row incl. the final linear's b_lin),
        # rows 101-127 == 0
        hb = [sing.tile([KP, (b - a) * 2 * NB], F16, name=f"hb{ci}")
              for ci, (a, b) in enumerate(CHUNKS)]
        outs = sing.tile([1, 4 * NB], F32)

        # PSUM pre-activation accumulators: per chunk, per block parity --
        # chunk c's TANH reads never share a PSUM tile/bank with another
        # chunk's matmul writes (no false serialization)
        pre = [[psum.tile([KP, (b - a) * NB], F32, name=f"pre{ci}_{q}")
                for q in range(2)]
               for ci, (a, b) in enumerate(CHUNKS)]
        lg = psum.tile([1, 2 * NB], F32)

        pre_v = [[p.rearrange("p (l s x) -> p l s x", l=b - a, s=TB)
                  for p in pre[ci]]
                 for ci, (a, b) in enumerate(CHUNKS)]
        hb_v = [t.rearrange("p (l q s x) -> p l q s x", l=b - a, q=2, s=TB)
                for t, (a, b) in zip(hb, CHUNKS)]

        # ---- prologue: loads (weights before bulk x so links can start) ----
        nc.gpsimd.dma_start(out=xt[0][:], in_=x_d[:, 0:xch])
        nc.gpsimd.dma_start(out=wih0[:], in_=wih0_d[:])
        nc.gpsimd.dma_start(
            out=wihr.rearrange("p (l j) -> p l j", l=L - 1),
            in_=wihr_d.rearrange("l p j -> p l j"),
        )
        nc.gpsimd.dma_start(
            out=whh.rearrange("p (l j) -> p l j", l=L),
            in_=whh_d.rearrange("l p j -> p l j"),
        )
        nc.gpsimd.dma_start(out=wlin[:], in_=wlin_d[:])
        for k in range(1, NXCH):
            nc.gpsimd.dma_start(out=xt[k][:], in_=x_d[:, k * xch:(k + 1) * xch])
        # rows 96-99 are overwritten by the first ACT writes; 100-127 stay 0
        # until the first ACT sets row 100 = tanh(20) = 1.0
        for t in hb:
            nc.vector.memset(t[96:KP, :], 0.0)
        # pre-zero PSUM once so first accumulates never meet NaN garbage
        for ci in range(len(CHUNKS)):
            for q in range(2):
                nc.vector.memset(pre[ci][q][:], 0.0)

        def gemm(l, t0):
            # input GEMM for layer l's block starting at t0.  Biases ride in
            # weight row 100 against h row 100 == 1.0; weight element
            # [100,100] == 20.0 regenerates the row (tanh(20) == 1.0).
            # start=False everywhere: start=True would clear has_written
            # BANK-wide, clobbering other layers in-flight; after the DVE
            # zero-fill, plain-write-or-add are both correct.
            q = (t0 // TB) % 2
            ci, li = chunk_of(l)
            outp = pre_v[ci][q][0:KP, li, :, :]
            nc.vector.memset(outp, 0.0)
            if l == 0:
                c0 = t0 * BC
                ch, off = c0 // xch, c0 % xch
                rhs = xt[ch][0:KP, off:off + NB]
                lhsT = wih0[:]
            else:
                pci, pli = chunk_of(l - 1)
                rhs = hb_v[pci][0:KP, pli, q, :, :]
                lhsT = wihr[:, (l - 1) * KP:l * KP]
            nc.tensor.matmul(outp, lhsT, rhs, start=False, stop=False,
                             skip_group_check=True)

        def mm_step(l, g):
            t = g - STAG * l
            if t == 0:
                return
            step, parity = g % TB, (g // TB) % 2
            pt = t - 1
            ci, li = chunk_of(l)
            h_src = hb_v[ci][0:KP, li, (pt // TB) % 2, pt % TB, :]
            outp = pre_v[ci][parity][0:KP, li, step, :]
            nc.tensor.matmul(outp, whh[:, l * KP:(l + 1) * KP], h_src,
                             start=False, stop=(step == TB - 1 or t == T - 1),
                             skip_group_check=True)

        def act_chunk(ci, ls, g):
            step, parity = g % TB, (g // TB) % 2
            a0 = CHUNKS[ci][0]
            a, b = ls[0] - a0, ls[-1] - a0
            src = pre_v[ci][parity][0:KP, a:b + 1, step, :]
            dst = hb_v[ci][0:KP, a:b + 1, parity, step, :]
            nc.scalar.activation(dst, src, TANH)

        def final_block(g):
            # runs when layer 9 has finished BOTH parities (every 16 links);
            # its two parity slots are contiguous in hb: 256 columns
            t9 = g - STAG * (L - 1)
            t0 = t9 - (2 * TB - 1)
            ci, li = chunk_of(L - 1)
            half = (t0 // (2 * TB)) % 2
            n2 = 2 * NB
            nc.tensor.matmul(lg[0:1, 0:n2], wlin[:, 0:1],
                             hb_v[ci][0:KP, li, :, :, :], start=True, stop=True,
                             skip_group_check=True)
            # sigmoid(z) = 0.5*(1 + tanh(z/2)) -- stays on the tanh table set
            nc.scalar.activation(outs[0:1, half * n2:(half + 1) * n2],
                                 lg[0:1, 0:n2], TANH, scale=0.5)
            nc.vector.tensor_scalar(outs[0:1, half * n2:(half + 1) * n2],
                                    outs[0:1, half * n2:(half + 1) * n2],
                                    0.5, 0.5,
                                    mybir.AluOpType.mult, mybir.AluOpType.add)
            nc.gpsimd.dma_start(out=out_d[0:1, t0 * BC:t0 * BC + n2],
                                in_=outs[0:1, half * n2:(half + 1) * n2])

        gemm(0, 0)

        for g in range(n_links):
            lmax = min(L - 1, g // STAG)
            lmin = max(0, -(-(g - (T - 1)) // STAG))
            active = list(range(lmin, lmax + 1))

            for ci in reversed(range(len(CHUNKS))):
                a, b = CHUNKS[ci]
                ls = [l for l in active if a <= l < b]
                if not ls:
                    continue
                for l in ls:
                    mm_step(l, g)
                act_chunk(ci, ls, g)
                if (ci == len(CHUNKS) - 1 and L - 1 in ls
                        and (g - STAG * (L - 1)) % (2 * TB) == 2 * TB - 1):
                    final_block(g)

            # off-chain input GEMMs for upcoming blocks, spread across links
            for l in range(L):
                s_l = 2 + (l % 6)
                if g % TB == s_l % TB:
                    t0 = g + TB - s_l - STAG * l
                    if 0 <= t0 <= T - TB and not (l == 0 and t0 == 0):
                        gemm(l, t0)

    nc._dbg = {"hb": hb, "pre": pre, "whh": whh, "xt": xt,
               "outs": outs, "lg": lg}
    if split_waits:
        _split_excess_waits(nc)
    return nc


def _get(T):
    if T not in _BUILD_CACHE:
        _BUILD_CACHE[T] = _build(T)
    return _BUILD_CACHE[T]


def _prep(x, W_ih, W_hh, b_ih, b_hh, W_lin, b_lin):
    T = x.shape[0]
    KP = 128
    bsum = (b_ih + b_hh).astype(np.float32)      # (L, H)
    wih0 = np.zeros((KP, KP), np.float32)
    wih0[0:H, 0:H] = W_ih[0].T
    wih0[H, 0:H] = bsum[0]
    wih0[H, H] = 20.0        # tanh(20) == 1.0 -> regenerates h row 100
    wihr = np.zeros((L - 1, KP, KP), np.float16)
    wihr[:, 0:H, 0:H] = W_ih[1:].transpose(0, 2, 1)
    wihr[:, H, 0:H] = bsum[1:]
    wihr[:, H, H] = 20.0
    whh = np.zeros((L, KP, KP), np.float16)
    whh[:, 0:H, 0:H] = W_hh.transpose(0, 2, 1)
    wlin = np.zeros((KP, 1), np.float16)
    wlin[0:H, 0] = W_lin[0]
    wlin[H, 0] = b_lin[0]
    in_maps = []
    for c in range(NCORES):
        xc = x[:, c * BC:(c + 1) * BC, :]          # (T, 16, 100)
        xct = xc.transpose(2, 0, 1).reshape(H, T * BC)
        xa = np.zeros((KP, T * BC), dtype=np.float32)
        xa[0:H] = xct
        xa[H] = 1.0
        in_maps.append({"x": xa, "wih0": wih0, "wihr": wihr,
                        "whh": whh, "wlin": wlin})
    return in_maps


def _run(inputs, trace=False, **kw):
    x = np.asarray(inputs["x"], dtype=np.float32)
    T = x.shape[0]
    nc = _get(T)
    in_maps = _prep(
        x,
        np.asarray(inputs["W_ih"], np.float32),
        np.asarray(inputs["W_hh"], np.float32),
        np.asarray(inputs["b_ih"], np.float32),
        np.asarray(inputs["b_hh"], np.float32),
        np.asarray(inputs["W_lin"], np.float32),
        np.asarray(inputs["b_lin"], np.float32),
    )
    res = run_bass_kernel_spmd(nc, in_maps, core_ids=list(range(NCORES)),
                               trace=trace, **kw)
    out = np.empty((T, B), dtype=np.float32)
    for c in range(NCORES):
        out[:, c * BC:(c + 1) * BC] = res.results[c]["out"].reshape(T, BC)
    return out.reshape(-1), res


def kernel(**inputs):
    out, _ = _run(inputs, trace=False)
    return out
